# revision 28
# baseline (speedup 1.0000x reference)
"""MoE adaptor layer (8 experts, gelu MLP + per-expert shortcut), trn2 x8.

Sharding: data-parallel, one batch row (2048 tokens) per core; every core
holds all 8 experts' weights (no collectives).

Per expert and token: h = gelu(x@W_in^T - c); out += g_e*(h@W_out^T +
x@W_sc^T + b_out). PE work dominates (~77 GFLOP/core), so:
  - mm1 (x@W_in^T, half the FLOPs) runs in fp8 e4m3 with
    MatmulPerfMode.DoubleRow (2 contraction k-tiles per instruction,
    2x bf16 throughput). x is scaled by 16, W_in by 1024 before the host
    cast; the 2^-14 descale plus the -c bias fold into the gelu
    activation for free. Measured end-to-end rel err 1.57e-2 (gate 2e-2);
    any additional fp8 GEMM (mm2 or shortcut) busts the error budget.
  - mm2 + shortcut stay bf16 and share one PSUM accumulation chain.
  - Everything cheap is precomputed on the host: all weights arrive
    pre-transposed and pre-cast (fp8/bf16), x arrives as x^T in both bf16
    and fp8, the gating softmax and the b_out/softmax-denominator init of
    the accumulator ship as inputs (gsc, acc0), like the baseline's neg_c.
  - Weight DMA is double-buffered one expert ahead, split across the SP
    and Activation hardware-DGE queues; a short chain of identity
    transposes warms the PE clock (p-state) while the first DMAs land.
"""
import sys

sys.path.insert(0, "/opt/trn_rl_repo")

import numpy as np
import ml_dtypes
from contextlib import ExitStack

# Problem constants (hardcoded per contract: kernel.py is self-contained).
B, S, D, O, M, E = 8, 2048, 768, 512, 1536, 8
T = S  # tokens per core (data-parallel over batch: 1 batch row per core)
P = 128
DT = D // P   # 6 d-tiles
MT = M // P   # 12 m-tiles
NT = T // P   # 16 token tiles per core
NCORES = 8

SX = 16.0     # fp8 scale for x
SW = 1024.0   # fp8 scale for W_in
E4NP = ml_dtypes.float8_e4m3
BFNP = ml_dtypes.bfloat16

import os
MM1_MODE = os.environ.get("MM1_MODE", "dr")  # dr | drx | fp8 | bf16
MM2_MODE = os.environ.get("MM2_MODE", "bf16")  # bf16 | fp8 (timing probe)

_CACHE = {}


def _build():
    import concourse.bass as bass
    import concourse.tile as tile
    from concourse import bacc, mybir
    from concourse.masks import make_identity

    f32 = mybir.dt.float32
    bf16 = mybir.dt.bfloat16
    fp8 = mybir.dt.float8e4
    AF = mybir.ActivationFunctionType
    ALU = mybir.AluOpType
    DR = mybir.MatmulPerfMode.DoubleRow

    nc = bacc.Bacc("TRN2", target_bir_lowering=False, debug=False,
                   num_devices=NCORES)

    mdt = {"dr": fp8, "drx": fp8, "fp8": fp8, "bf16": bf16}[MM1_MODE]

    # All operands arrive pre-transposed / pre-cast from the host.
    # Gating softmax and the b_out init are token-cheap and precomputed
    # host-side: gsc = softmax(x @ w_gate) in [p, nt, e] layout, acc0 =
    # gsc @ b_out.
    xt8_d = nc.dram_tensor("xt8", (D, T), mdt, kind="ExternalInput").ap()
    xt8lo_d = (nc.dram_tensor("xt8lo", (D, T), mdt, kind="ExternalInput").ap()
               if MM1_MODE == "drx" else None)
    xtb_d = nc.dram_tensor("xtb", (D, T), bf16, kind="ExternalInput").ap()
    win_d = nc.dram_tensor("win8", (E, D, M), mdt, kind="ExternalInput").ap()
    wout_d = nc.dram_tensor("woutT", (E, M, O),
                            fp8 if MM2_MODE == "fp8" else bf16,
                            kind="ExternalInput").ap()
    wsc_d = nc.dram_tensor("wscT", (E, D, O), bf16, kind="ExternalInput").ap()
    negc_d = nc.dram_tensor("negcT", (P, MT, E), f32, kind="ExternalInput").ap()
    gsc_d = nc.dram_tensor("gsc", (P, NT, E), f32, kind="ExternalInput").ap()
    acc0_d = nc.dram_tensor("acc0", (T, O), bf16, kind="ExternalInput").ap()
    out_d = nc.dram_tensor("out", (T, O), f32, kind="ExternalOutput").ap()

    with tile.TileContext(nc) as tc, ExitStack() as ctx:
        const = ctx.enter_context(tc.tile_pool(name="const", bufs=1))
        wp = ctx.enter_context(tc.tile_pool(name="wp", bufs=2))
        comb = ctx.enter_context(tc.tile_pool(name="comb", bufs=2))
        pmm1 = ctx.enter_context(tc.tile_pool(name="pmm1", bufs=5, space="PSUM"))
        pmm2 = ctx.enter_context(tc.tile_pool(name="pmm2", bufs=2, space="PSUM"))
        ptr = ctx.enter_context(tc.tile_pool(name="ptr", bufs=1, space="PSUM"))

        ident = const.tile([P, P], bf16)
        make_identity(nc, ident)

        # ---- persistent SBUF tensors ----
        xT = const.tile([P, DT, T], bf16)        # x^T
        xT8 = xT if MM1_MODE == "bf16" else const.tile([P, DT, T], mdt)
        xT8lo = (const.tile([P, DT, T], mdt, name="xT8lo")
                 if MM1_MODE == "drx" else None)
        hdt = fp8 if MM2_MODE == 'fp8' else bf16
        hT = const.tile([P, MT, T], hdt)         # gelu output, full T
        accs = [const.tile([P, 4, O], f32, name=f"acc{c}")
                for c in range(4)]               # output accumulator
        gsc = const.tile([P, NT, E], f32)        # normalized gates [t%128, tt, e]
        negcT = const.tile([P, MT, E], f32)      # [m_in, mt, e]

        def load_weights(e):
            winT8 = wp.tile([P, DT, M], mdt, tag="win")
            woutT = wp.tile([P, MT, O], fp8 if MM2_MODE == "fp8" else bf16,
                            tag="wout")
            wscT = wp.tile([P, DT, O], bf16, tag="wsc")
            nc.sync.dma_start(winT8, win_d[e].rearrange("(dt p) m -> p dt m", p=P))
            nc.scalar.dma_start(woutT, wout_d[e].rearrange("(mt p) o -> p mt o", p=P))
            nc.sync.dma_start(wscT, wsc_d[e].rearrange("(dt p) o -> p dt o", p=P))
            return winT8, woutT, wscT

        # ---- input DMAs, ordered so e0 mm1 can start earliest ----
        # Critical path for the first mm1 group: xt8[:, :, :512] (sync) and
        # the first m-half of win8[0] (scalar) land in parallel (~5 us);
        # everything else follows behind on its queue.
        nc.gpsimd.dma_start(negcT, negc_d)
        xt8_v = None
        if MM1_MODE != "bf16":
            xt8_v = xt8_d.rearrange("(dt p) t -> p dt t", p=P)
            nc.sync.dma_start(xT8[:, :, :512], xt8_v[:, :, :512])
        winT8_0 = wp.tile([P, DT, M], mdt, tag="win")
        woutT_0 = wp.tile([P, MT, O], fp8 if MM2_MODE == "fp8" else bf16,
                          tag="wout")
        wscT_0 = wp.tile([P, DT, O], bf16, tag="wsc")
        win0_v = win_d[0].rearrange("(dt p) m -> p dt m", p=P)
        nc.scalar.dma_start(winT8_0[:, :, :M // 2], win0_v[:, :, :M // 2])
        nc.scalar.dma_start(winT8_0[:, :, M // 2:], win0_v[:, :, M // 2:])
        if MM1_MODE != "bf16":
            nc.sync.dma_start(xT8[:, :, 512:1024], xt8_v[:, :, 512:1024])
            nc.sync.dma_start(xT8[:, :, 1024:], xt8_v[:, :, 1024:])
        if MM1_MODE == "drx":
            nc.scalar.dma_start(xT8lo, xt8lo_d.rearrange("(dt p) t -> p dt t", p=P))
        nc.gpsimd.dma_start(gsc, gsc_d)
        w0 = (winT8_0, woutT_0, wscT_0)

        # warm the PE clock while the first DMAs land (p-state ramps after
        # ~3 us of sustained busy); results are discarded. 8 transposes
        # cover the ramp at the low clock without delaying mm1 if the
        # DMAs land early.
        for i in range(8):
            pw = ptr.tile([P, 4, P], bf16, tag="gtr")
            nc.tensor.transpose(pw[:, i % 4, :], ident, ident)

        act_scale = 1.0 / (SX * SW) if MM1_MODE != "bf16" else 1.0

        def mm1_q(e, winT8, tq):
            """h^T for 512 tokens: 12 m-tiles, 3 DoubleRow matmuls each."""
            t0 = tq * 512
            for mt in range(MT):
                ph = pmm1.tile([P, O], f32, tag="mm1")
                if MM1_MODE in ("dr", "drx"):
                    nk = 6 if MM1_MODE == "drx" else 3
                    for k in range(nk):
                        xop = xT8 if k < 3 else xT8lo
                        kk = k % 3
                        nc.tensor.matmul(
                            ph, winT8[:, 2 * kk:2 * kk + 2, mt * P:(mt + 1) * P],
                            xop[:, 2 * kk:2 * kk + 2, t0:t0 + 512],
                            start=(k == 0), stop=(k == nk - 1), perf_mode=DR)
                else:
                    for k in range(DT):
                        nc.tensor.matmul(
                            ph, winT8[:, k, mt * P:(mt + 1) * P],
                            xT8[:, k, t0:t0 + 512],
                            start=(k == 0), stop=(k == DT - 1))
                nc.scalar.activation(hT[:, mt, t0:t0 + 512], ph, AF.Gelu,
                                     bias=negcT[:, mt, e:e + 1],
                                     scale=act_scale)

        def mm2_t(e, woutT, wscT, tg):
            """one [128-token, 512] output tile: h@W_out^T + x@W_sc^T."""
            po = pmm2.tile([P, O], f32, tag="mm2")
            if MM2_MODE == "fp8":
                for k in range(6):
                    nc.tensor.matmul(
                        po, hT[:, 2 * k:2 * k + 2, tg * P:(tg + 1) * P],
                        woutT[:, 2 * k:2 * k + 2, :],
                        start=(k == 0), stop=False, perf_mode=DR)
            else:
                for mt in range(MT):
                    nc.tensor.matmul(po, hT[:, mt, tg * P:(tg + 1) * P],
                                     woutT[:, mt, :],
                                     start=(mt == 0), stop=False)
            for dt_ in range(DT):
                nc.tensor.matmul(po, xT[:, dt_, tg * P:(tg + 1) * P],
                                 wscT[:, dt_, :],
                                 start=False, stop=(dt_ == DT - 1))
            av = accs[tg // 4][:, tg % 4, :]
            nc.vector.scalar_tensor_tensor(
                out=av, in0=po, scalar=gsc[:, tg, e:e + 1],
                in1=av, op0=ALU.mult, op1=ALU.add)
            if e == E - 1:
                # split so the final tile's store drains in half the time
                nc.scalar.dma_start(out_d[tg * P:tg * P + 64, :], av[:64, :])
                nc.scalar.dma_start(out_d[tg * P + 64:(tg + 1) * P, :],
                                    av[64:, :])

        # ---- expert pipeline ----
        # e0's first two mm1 quarters are emitted BEFORE the non-critical
        # startup DMAs: those dma_starts sit behind tq0/tq1's activations
        # in their queues, so their dispatch (and DMA-engine slot use) is
        # deferred until the mm1-critical xt8/win8 transfers are done.
        mm1_q(0, w0[0], 0)
        mm1_q(0, w0[0], 1)
        wout0_v = wout_d[0].rearrange("(mt p) o -> p mt o", p=P)
        nc.scalar.dma_start(woutT_0[:, :MT // 2, :], wout0_v[:, :MT // 2, :])
        nc.scalar.dma_start(woutT_0[:, MT // 2:, :], wout0_v[:, MT // 2:, :])
        nc.sync.dma_start(wscT_0, wsc_d[0].rearrange("(dt p) o -> p dt o", p=P))
        # xT quarters and bf16 acc-init chunks in need-time order (mm2
        # consumes token tiles left to right over ~40 us)
        xtb_v = xtb_d.rearrange("(dt p) t -> p dt t", p=P)
        acc0_v = acc0_d.rearrange("(nt p) o -> p nt o", p=P)
        for c in range(4):
            nc.scalar.dma_start(xT[:, :, 512 * c:512 * (c + 1)],
                                xtb_v[:, :, 512 * c:512 * (c + 1)])
            ab = comb.tile([P, 4, O], bf16, tag="a0")
            nc.gpsimd.dma_start(ab, acc0_v[:, 4 * c:4 * (c + 1), :])
            nc.vector.tensor_copy(accs[c], ab)

        winT8, woutT, wscT = w0
        for e in range(E):
            if e + 1 < E:
                nw = load_weights(e + 1)
            # interleave so mm2 never waits on the gelu of its own tokens
            if e > 0:
                mm1_q(e, winT8, 0)
                mm1_q(e, winT8, 1)
            for tg in range(4):
                mm2_t(e, woutT, wscT, tg)
            mm1_q(e, winT8, 2)
            for tg in range(4, 8):
                mm2_t(e, woutT, wscT, tg)
            mm1_q(e, winT8, 3)
            for tg in range(8, NT):
                mm2_t(e, woutT, wscT, tg)
            if e + 1 < E:
                winT8, woutT, wscT = nw

    nc.compile()
    return nc


def _get_nc():
    if "nc" not in _CACHE:
        _CACHE["nc"] = _build()
    return _CACHE["nc"]


def _q8(a, scale):
    return np.clip(np.asarray(a, np.float32) * scale, -240, 240).astype(E4NP)


def prep_in_maps(x, w_gate, bias_in, W_in, W_out, b_out, W_sc):
    x = np.asarray(x, np.float32)
    W_in = np.asarray(W_in, np.float32)
    negc = -np.einsum("ed,emd->em", np.asarray(bias_in, np.float64),
                      np.asarray(W_in, np.float64)).astype(np.float32)
    negcT = np.ascontiguousarray(negc.T.reshape(MT, P, E).transpose(1, 0, 2))
    if MM1_MODE == "bf16":
        win_h = np.ascontiguousarray(W_in.transpose(0, 2, 1).astype(BFNP))
    else:
        win_h = np.ascontiguousarray(_q8(W_in.transpose(0, 2, 1), SW))
    shared = {
        "negcT": negcT,
        "win8": win_h,
        "woutT": (np.ascontiguousarray(
            _q8(np.asarray(W_out, np.float32).transpose(0, 2, 1), SW))
            if MM2_MODE == "fp8" else np.ascontiguousarray(
            np.asarray(W_out, np.float32).transpose(0, 2, 1).astype(BFNP))),
        "wscT": np.ascontiguousarray(
            np.asarray(W_sc, np.float32).transpose(0, 2, 1).astype(BFNP)),
    }
    wgf = np.asarray(w_gate, np.float32)
    bof = np.asarray(b_out, np.float32)
    in_maps = []
    for i in range(NCORES):
        xt = np.ascontiguousarray(x[i].T)
        logits = (x[i].astype(BFNP).astype(np.float32)
                  @ wgf.astype(BFNP).astype(np.float32))
        ex = np.exp(logits)
        g = (ex / ex.sum(axis=1, keepdims=True)).astype(np.float32)  # [T, E]
        acc0 = (g @ bof).astype(BFNP)                                # [T, O]
        gsc = np.ascontiguousarray(
            g.reshape(NT, P, E).transpose(1, 0, 2))
        im_x = {}
        if MM1_MODE == "drx":
            x8 = _q8(xt, SX)
            xlo = xt - x8.astype(np.float32) / SX
            im_x = {"xt8lo": _q8(xlo, SX)}
        in_maps.append({
            "xt8": xt.astype(BFNP) if MM1_MODE == "bf16" else _q8(xt, SX),
            "xtb": xt.astype(BFNP),
            **im_x,
            "gsc": gsc,
            "acc0": acc0,
            **shared,
        })
    return in_maps


def kernel(x, w_gate, bias_in, W_in, W_out, b_out, W_sc):
    from concourse.bass_utils import run_bass_kernel_spmd

    nc = _get_nc()
    in_maps = prep_in_maps(x, w_gate, bias_in, W_in, W_out, b_out, W_sc)
    res = run_bass_kernel_spmd(nc, in_maps, core_ids=list(range(NCORES)))
    out = np.stack([res.results[i]["out"] for i in range(NCORES)], axis=0)
    return out.astype(np.float32)
